# revision 4
# baseline (speedup 1.0000x reference)
"""CPD block (1x1 conv -> depthwise 1x3 -> depthwise 3x1 + bias) on 8 trn2 cores.

Contract: kernel(**inputs) takes FULL inputs (x:[8,64,256,256] f32, w1:[64,64],
wh:[64,3], wv:[64,3], bias:[64]) and returns the FULL output [8,64,256,256] f32.

Strategy
--------
Data-parallel over batch: 1 image per core, 8 cores, no collectives.

The input is zero-padded on the host to fp16 [128, 130, 258] per core: the two
128-row image halves (with a 1-row halo each side) are stacked on the 128 SBUF
partitions (partition p = 2*c + hh), so DMA and the vector engines run at full
128-partition width.

PE: the 1x1 conv and the horizontal 1x3 depthwise conv are fused into 3 "tap"
matmuls over the in-channel dim (W_dx[o,c] = w1[o,c]*wh[o,dx]) accumulated in
PSUM; the taps read column-shifted views of the padded x tile.  Each tap uses
K=128 block-diagonal weights (diag(W_dx, W_dx)) so one N=512 instruction
computes both halves at once.  Matmuls run in fp16 (~5e-4 rel err).  Each z
row is computed exactly ONCE: 16-row segments with a 2-row carry copied from
the previous segment's tile instead of halo recompute.

ACT evacuates z from PSUM to SBUF as fp16 (raw), and computes the wv2 tap
(Ch = z*wv2, Copy-with-scale) for 7 of 8 segments to offload DVE.

DVE does the vertical 3x1 conv + bias with fp16 fast modes:
    A  = z[k]   * wv0          (tensor_scalar, 4x mode)
    Bh = z[k+1] * wv1 + bias   (tensor_scalar two-scalar, 4x)
    Ch = z[k+2] * wv2          (tensor_scalar 4x; on ACT for 7/8 segments)
    U  = A + Bh                (tensor_tensor, 2x)
    ot = U + Ch                (tensor_tensor, 2x)

The output is written to DRAM as fp16 (halves the write traffic; tolerance is
2e-2, fp16 adds ~5e-4) and upcast to f32 on the host.
"""

import numpy as np

import concourse.bacc as bacc
import concourse.mybir as mybir
from concourse.tile import TileContext
from concourse.bass_utils import run_bass_kernel_spmd

B, C, O = 8, 64, 64
H, W = 256, 256
WP = W + 2             # padded width
N_CORES = 8
HALF = H // 2          # rows per half-image
SEG = 16               # output rows per half per segment
NSEG = HALF // SEG
ZB = 8                 # z rows per PSUM tile (4 banks)

F16 = mybir.dt.float16

# Segments whose Ch (wv2 tap scale) runs on ACT instead of DVE.
ACH_DEFAULT = (0, 1, 2, 3, 4, 5, 6)


def _emit_rep(tc, pools, orr, x, w_sb, v_sb, ach):
    """Emit one full-image rep (8 segments) of the kernel body."""
    nc = tc.nc
    f32 = mybir.dt.float32
    mult, add = mybir.AluOpType.mult, mybir.AluOpType.add
    Copy = mybir.ActivationFunctionType.Copy
    xpool, opool, zspool, tpool, zpool = pools
    ZR = SEG + 2

    zs_prev = None
    for s in range(NSEG):
        r0 = s * SEG
        # zs row m <-> z half-local row 16s-1+m, m in [0, 18).
        # Fresh z rows this seg: seg0 all 18, else rows [2, 18).
        zs = zspool.tile([128, ZR, W], F16, tag="zs")
        fresh0 = 0 if s == 0 else 2
        nx = ZR - fresh0
        xt = xpool.tile([128, ZR, WP], F16, tag="xt")
        # DRAM x row j = z half-local row j-1; fresh z rows are
        # [16s+fresh0-1, 16s+17) -> DRAM rows [16s+fresh0, 16s+18).
        nc.sync.dma_start(
            out=xt[:, 0:nx, :],
            in_=x[:, r0 + fresh0 : r0 + ZR, :],
        )
        if s > 0:
            # carry z rows 16s-1, 16s from the previous segment's tile
            nc.vector.tensor_copy(out=zs[:, 0:2, :], in_=zs_prev[:, 16:18, :])

        for b0 in range(fresh0, ZR, ZB):
            zb = min(ZB, ZR - b0)
            zt = zpool.tile([128, ZB * W], f32, tag="zt")
            for j in range(zb // 2):  # 2-row chunks (one PSUM bank)
                xr = b0 - fresh0 + 2 * j
                for i, dx in enumerate((0, 1, 2)):
                    nc.tensor.matmul(
                        out=zt[:, j * 512 : (j + 1) * 512],
                        lhsT=w_sb[:, dx * 128 : (dx + 1) * 128],
                        rhs=xt[:, xr : xr + 2, dx : dx + W],
                        start=(i == 0),
                        stop=(i == 2),
                    )
            nc.scalar.copy(
                out=zs[:, b0 : b0 + zb, :],
                in_=zt.rearrange("p (r w) -> p r w", w=W)[:, :zb, :],
            )

        # Vertical conv + bias over the 16 output rows of this segment.
        at = tpool.tile([128, SEG, W], F16, tag="at")
        bt = tpool.tile([128, SEG, W], F16, tag="bt")
        ct = tpool.tile([128, SEG, W], F16, tag="ct")
        ut = tpool.tile([128, SEG, W], F16, tag="ut")
        ot = opool.tile([128, SEG, W], F16, tag="ot")

        nc.vector.tensor_scalar(
            out=at, in0=zs[:, 0:SEG, :],
            scalar1=v_sb[:, 0:1], scalar2=None, op0=mult,
        )
        nc.vector.tensor_scalar(
            out=bt, in0=zs[:, 1 : SEG + 1, :],
            scalar1=v_sb[:, 1:2], scalar2=v_sb[:, 3:4],
            op0=mult, op1=add,
        )
        if s in ach:
            nc.scalar.activation(
                out=ct, in_=zs[:, 2 : SEG + 2, :], func=Copy,
                scale=v_sb[:, 2:3],
            )
        else:
            nc.vector.tensor_scalar(
                out=ct, in0=zs[:, 2 : SEG + 2, :],
                scalar1=v_sb[:, 2:3], scalar2=None, op0=mult,
            )
        nc.vector.tensor_tensor(out=ut, in0=at, in1=bt, op=add)
        nc.vector.tensor_tensor(out=ot, in0=ut, in1=ct, op=add)

        nc.scalar.dma_start(out=orr[:, :, r0 : r0 + SEG, :], in_=ot)
        zs_prev = zs


def _kernel_body(tc, out, x, w, v, reps=1, ach=ACH_DEFAULT, warmup=0,
                 hw_loop=0):
    nc = tc.nc
    f32 = mybir.dt.float32

    with (
        tc.tile_pool(name="const", bufs=1) as cpool,
        tc.tile_pool(name="xp", bufs=3) as xpool,
        tc.tile_pool(name="op", bufs=2) as opool,
        tc.tile_pool(name="zs", bufs=2) as zspool,
        tc.tile_pool(name="tp", bufs=2) as tpool,
        tc.tile_pool(name="zp", bufs=2, space="PSUM") as zpool,
    ):
        w_sb = cpool.tile([128, 3 * 128], F16)
        nc.sync.dma_start(out=w_sb, in_=w)
        v_sb = cpool.tile([128, 4], f32)
        nc.sync.dma_start(out=v_sb, in_=v)

        if warmup:
            # Dummy matmuls while the first segment DMA is in flight: ramps
            # the PE clock gate before the real work starts.
            wz = zpool.tile([128, ZB * W], f32, tag="zt")
            for i in range(warmup):
                nc.tensor.matmul(
                    out=wz[:, 0:384],
                    lhsT=w_sb[:, 0:128],
                    rhs=w_sb,
                    start=(i == 0),
                    stop=(i == warmup - 1),
                )

        # out viewed as [c, hh, hr, w]: flat order matches the ot tiles
        # [128=(2c+hh), SEG, W] exactly (partition p = 2c+hh).
        orr = out.rearrange("c (hh hr) w -> c hh hr w", hh=2)
        pools = (xpool, opool, zspool, tpool, zpool)

        if hw_loop:
            with tc.For_i(0, hw_loop):
                _emit_rep(tc, pools, orr, x, w_sb, v_sb, ach)
        else:
            for _ in range(reps):
                _emit_rep(tc, pools, orr, x, w_sb, v_sb, ach)


_CACHE = {}


def _build(reps=1, ach=ACH_DEFAULT, warmup=0, hw_loop=0):
    key = (reps, tuple(ach), warmup, hw_loop)
    if key in _CACHE:
        return _CACHE[key]
    nc = bacc.Bacc("TRN2", target_bir_lowering=False, debug=False)
    xd = nc.dram_tensor("x", [128, HALF + 2, WP], F16, kind="ExternalInput").ap()
    wd = nc.dram_tensor("w", [128, 3 * 128], F16, kind="ExternalInput").ap()
    vd = nc.dram_tensor("v", [128, 4], mybir.dt.float32, kind="ExternalInput").ap()
    od = nc.dram_tensor("out", [C, H, W], F16, kind="ExternalOutput").ap()
    with TileContext(nc) as tc:
        _kernel_body(tc, od, xd, wd, vd, reps=reps, ach=ach, warmup=warmup,
                     hw_loop=hw_loop)
    nc.compile()
    _CACHE[key] = nc
    return nc


def prep_inputs(x, w1, wh, wv, bias):
    """Host-side input prep shared by kernel() and benchmarks."""
    x = np.asarray(x, dtype=np.float32)
    w1 = np.asarray(w1, dtype=np.float32)
    wh = np.asarray(wh, dtype=np.float32)
    wv = np.asarray(wv, dtype=np.float32)
    bias = np.asarray(bias, dtype=np.float32)

    # Host-side zero pad, then split into two 128-row halves (with one halo
    # row on each side) stacked on the partition axis: [B, 128, HALF+2, WP].
    xpad = np.zeros((B, C, H + 2, WP), np.float16)
    xpad[:, :, 1 : H + 1, 1 : W + 1] = x.astype(np.float16)
    xp = np.empty((B, C, 2, HALF + 2, WP), np.float16)
    for hh in range(2):
        xp[:, :, hh] = xpad[:, :, hh * HALF : hh * HALF + HALF + 2, :]
    xp = xp.reshape(B, 128, HALF + 2, WP)  # partition p = 2*c + hh

    # Fold the horizontal conv into the 1x1 and build K=128 block-diagonal
    # taps: lhsT_dx = diag(W_dx.T, W_dx.T) with W_dx[o,c] = w1[o,c]*wh[o,dx].
    w_np = np.zeros((128, 3 * 128), np.float16)
    for dx in range(3):
        blk = (w1 * wh[:, dx : dx + 1]).T.astype(np.float16)  # [c, o]
        wb = np.zeros((C, 2, O, 2), np.float16)
        wb[:, 0, :, 0] = blk
        wb[:, 1, :, 1] = blk
        w_np[:, dx * 128 : (dx + 1) * 128] = wb.reshape(128, 128)
    # Per-partition vertical-tap weights + bias: [wv0, wv1, wv2, bias]
    v_np = np.stack([wv[:, 0], wv[:, 1], wv[:, 2], bias], axis=1)
    v_np = np.repeat(v_np, 2, axis=0).astype(np.float32)  # p = 2*o + hh
    return xp, w_np, v_np


def kernel(x, w1, wh, wv, bias, _results_out=None):
    xp, w_np, v_np = prep_inputs(x, w1, wh, wv, bias)
    nc = _build()
    in_maps = [{"x": xp[b], "w": w_np, "v": v_np} for b in range(B)]
    res = run_bass_kernel_spmd(nc, in_maps, list(range(N_CORES)))
    if _results_out is not None:
        _results_out.append(res)
    return np.stack(
        [res.results[b]["out"].astype(np.float32) for b in range(B)], axis=0
    )
